# revision 7
# baseline (speedup 1.0000x reference)
"""CondConv2d (MoE routed conv) Trainium2 kernel.

Math: out[b] = sum_e routing[b,e] * conv3x3(x[b], W[e])
Since the expert mix is linear in W, this equals
    out[b] = conv3x3(x[b], Wmix_b),  Wmix_b = sum_e routing[b,e] * W[e]
which needs 1 conv per sample instead of E=4 (4x less PE work).

Sharding: data-parallel over batch, B=16 -> 2 samples per core on 8 cores.
Weights (all 4 experts, transposed to [ci, e, tap, co] on host) are
replicated; the per-sample mix happens on-device on the Vector engine.

Conv as implicit GEMM: x is zero-padded on host to [ci, 58, 58]; for each
of 9 taps the matmul streams a shifted window of the padded image
(rhs = xpad[:, blk*8+kh : +8, kw : kw+56], N=448) against the tap's mixed
weight slice (lhsT = Wmix[ci, co], K=ci on partitions), accumulating all
9 taps into one PSUM bank. 7 row-blocks of 8 rows cover the 56 output
rows. Matmuls run as float32r (1 cycle/row at N>=256 vs 4 for fp32);
fp32r is fp32 with the mantissa rounded to 11 bits (low 12 bits zero),
~16x more accurate than bf16. x is pre-rounded to fp32r on the host; the
weight mix is rounded by the DVE output cast.
"""

import os
import sys

os.environ.setdefault("MYCRO_LOCAL_CACHE", "1")
for _p in ("/opt/trn_rl_repo",):
    if _p not in sys.path:
        sys.path.insert(0, _p)

import numpy as np

B, CIN, COUT, H, W_SP = 16, 128, 128, 56, 56
E, KH, KW = 4, 3, 3
NCORES = 8
SPC = B // NCORES          # samples per core
HP, WP = H + 2, W_SP + 2   # padded spatial
NTAP = KH * KW
RPB = 8                    # output rows per matmul block
NBLK = H // RPB
NT = RPB * W_SP            # moving-operand free size per matmul (448)

_cached_nc = None


def _round_f32r(a):
    """Round fp32 array to fp32r bits (RNE to 11 mantissa bits)."""
    u = a.view(np.uint32)
    lsb = (u >> np.uint32(12)) & np.uint32(1)
    return ((u + np.uint32(0x7FF) + lsb) & np.uint32(0xFFFFF000)).view(np.float32)


def _build_nc():
    import concourse.tile as tile
    from concourse import bacc, mybir

    f32 = mybir.dt.float32
    f32r = mybir.dt.float32r

    nc = bacc.Bacc(
        "TRN2", target_bir_lowering=False, debug=False, num_devices=NCORES
    )

    xpad_d = nc.dram_tensor(
        "xpad", [SPC, CIN, HP * WP], f32r, kind="ExternalInput"
    ).ap()
    wt_d = nc.dram_tensor(
        "wt", [CIN, E * NTAP * COUT], f32, kind="ExternalInput"
    ).ap()
    rb_d = nc.dram_tensor("rb", [128, SPC * E], f32, kind="ExternalInput").ap()
    out_d = nc.dram_tensor(
        "out", [SPC, COUT, H * W_SP], f32, kind="ExternalOutput"
    ).ap()

    with tile.TileContext(nc) as tc:
        with (
            tc.tile_pool(name="wt", bufs=1) as wtp,
            tc.tile_pool(name="rb", bufs=1) as rbp,
            tc.tile_pool(name="x", bufs=2) as xpool,
            tc.tile_pool(name="wmix", bufs=2) as wmp,
            tc.tile_pool(name="ob", bufs=2) as opool,
            tc.tile_pool(name="ps", bufs=8, space="PSUM") as pspool,
        ):
            rb_t = rbp.tile([128, SPC * E], f32)
            nc.sync.dma_start(rb_t[:], rb_d[:])
            wt_t = wtp.tile([CIN, E * NTAP * COUT], f32)
            # one DMA per expert so the first mix ops can start early
            for e in range(E):
                sl = slice(e * NTAP * COUT, (e + 1) * NTAP * COUT)
                nc.sync.dma_start(wt_t[:, sl], wt_d[:, sl])

            for s in range(SPC):
                xp = xpool.tile([CIN, HP * WP], f32r)
                nc.sync.dma_start(xp[:], xpad_d[s])

                # Wmix = sum_e routing[s, e] * WT[e]   on DVE (output cast
                # rounds to fp32r as the FP32r matmul verifier requires)
                wm = wmp.tile([CIN, NTAP * COUT], f32r)
                nc.vector.tensor_scalar_mul(
                    wm[:], wt_t[:, 0 : NTAP * COUT], rb_t[:, s * E : s * E + 1]
                )
                for e in range(1, E):
                    nc.vector.scalar_tensor_tensor(
                        wm[:],
                        wt_t[:, e * NTAP * COUT : (e + 1) * NTAP * COUT],
                        rb_t[:, s * E + e : s * E + e + 1],
                        wm[:],
                        mybir.AluOpType.mult,
                        mybir.AluOpType.add,
                    )

                ob = opool.tile([COUT, H * W_SP], f32)
                xp3 = xp[:].rearrange("p (h w) -> p h w", h=HP)
                for blk in range(NBLK):
                    ps = pspool.tile([COUT, NT], f32)
                    for kh in range(KH):
                        for kw in range(KW):
                            tap = kh * KW + kw
                            rhs = xp3[
                                :,
                                blk * RPB + kh : blk * RPB + kh + RPB,
                                kw : kw + W_SP,
                            ]
                            nc.tensor.matmul(
                                ps[:],
                                wm[:, tap * COUT : (tap + 1) * COUT],
                                rhs,
                                start=(tap == 0),
                                stop=(tap == NTAP - 1),
                            )
                    nc.vector.tensor_copy(ob[:, blk * NT : (blk + 1) * NT], ps[:])
                nc.sync.dma_start(out_d[s], ob[:])

    nc.compile()
    return nc


def _get_nc():
    global _cached_nc
    if _cached_nc is None:
        _cached_nc = _build_nc()
    return _cached_nc


def _prep_inputs(x, routing_weights, W):
    x = np.ascontiguousarray(x, dtype=np.float32)
    routing_weights = np.ascontiguousarray(routing_weights, dtype=np.float32)
    W = np.ascontiguousarray(W, dtype=np.float32)

    xpad = np.zeros((B, CIN, HP, WP), np.float32)
    xpad[:, :, 1 : H + 1, 1 : W_SP + 1] = _round_f32r(x.reshape(B, CIN, H, W_SP))
    xpad = xpad.reshape(B, CIN, HP * WP)

    # W[e, co, ci, kh, kw] -> wt[ci, (e, kh, kw, co)]
    wt = np.ascontiguousarray(np.transpose(W, (2, 0, 3, 4, 1))).reshape(
        CIN, E * NTAP * COUT
    )

    in_maps = []
    for c in range(NCORES):
        r = routing_weights[c * SPC : (c + 1) * SPC]  # [SPC, E]
        rb = np.ascontiguousarray(
            np.broadcast_to(r.reshape(1, SPC * E), (128, SPC * E))
        )
        in_maps.append(
            {
                "xpad": xpad[c * SPC : (c + 1) * SPC],
                "wt": wt,
                "rb": rb,
            }
        )
    return in_maps


def _run(in_maps, **kwargs):
    from concourse import bass_utils

    nc = _get_nc()
    res = bass_utils.run_bass_kernel_spmd(
        nc, in_maps, core_ids=list(range(NCORES)), **kwargs
    )
    out = np.concatenate(
        [res.results[c]["out"] for c in range(NCORES)], axis=0
    ).reshape(B, COUT, H, W_SP)
    return out, res


def kernel(x, routing_weights, W):
    in_maps = _prep_inputs(x, routing_weights, W)
    out, _ = _run(in_maps)
    return out


# revision 12
# speedup vs baseline: 1.1393x; 1.1393x over previous
"""CondConv2d (MoE routed conv) Trainium2 kernel.

Math: out[b] = sum_e routing[b,e] * conv3x3(x[b], W[e])
Since the expert mix is linear in W, this equals
    out[b] = conv3x3(x[b], Wmix_b),  Wmix_b = sum_e routing[b,e] * W[e]
which needs 1 conv per sample instead of E=4 (4x less PE work).

Sharding: data-parallel over batch, B=16 -> 2 samples per core on 8 cores.
Weights (all 4 experts, transposed to [ci, tap, e, co] on host) are
replicated; the per-sample mix happens on-device on the Vector engine.

Conv as implicit GEMM: x is zero-padded on host to [ci, 58, 58]; for each
of 9 taps the matmul streams a shifted window of the padded image
(rhs = xpad[:, blk*8+kh : +8, kw : kw+56], N=448) against the tap's mixed
weight slice (lhsT = Wmix[ci, co], K=ci on partitions), accumulating all
9 taps into one PSUM bank. 7 row-blocks of 8 rows cover the 56 output
rows. Matmuls run as float32r (1 cycle/row at N>=256 vs 4 for fp32);
fp32r is fp32 with the mantissa rounded to 11 bits, ~16x more accurate
than bf16. x is pre-rounded to fp32r on the host; the weight mix is
rounded by the DVE output cast.

Schedule: sample 0 runs tap-outer (all 7 PSUM banks accumulate one tap at
a time) so matmuls start after only the first tap's weights + first x
rows arrive; its loads are chunked and interleaved on the sync DMA ring.
Sample 1 runs block-outer (9 taps into one bank, then drain) so the
output streams out incrementally and the kernel tail is one small store.
Dummy matmuls on a zeroed tile during the load phase keep the PE HAM
clock-gate warm (2.4 GHz) for the real stream. Stores go out on the
scalar-engine DMA ring to stay off the load path.
"""

import os
import sys

os.environ.setdefault("MYCRO_LOCAL_CACHE", "1")
for _p in ("/opt/trn_rl_repo",):
    if _p not in sys.path:
        sys.path.insert(0, _p)

import numpy as np

B, CIN, COUT, H, W_SP = 16, 128, 128, 56, 56
E, KH, KW = 4, 3, 3
NCORES = 8
SPC = B // NCORES          # samples per core
HP, WP = H + 2, W_SP + 2   # padded spatial
NTAP = KH * KW
RPB = 8                    # output rows per matmul block
NBLK = H // RPB
NT = RPB * W_SP            # moving-operand free size per matmul (448)
N_WARM = 12                # HAM warm-up dummy matmuls

# x chunks (padded-row ranges); block b needs padded rows [8b, 8b+10)
XCH = [(0, 26), (24, 18), (40, 18)]        # (start_row, n_rows)
BLK_CH = [0, 0, 0, 1, 1, 2, 2]             # block -> chunk

_cached_nc = None


def _round_f32r(a):
    """Round fp32 array to fp32r bits (RNE to 11 mantissa bits)."""
    u = a.view(np.uint32)
    lsb = (u >> np.uint32(12)) & np.uint32(1)
    return ((u + np.uint32(0x7FF) + lsb) & np.uint32(0xFFFFF000)).view(np.float32)


def _build_nc():
    import concourse.tile as tile
    from concourse import bacc, mybir

    f32 = mybir.dt.float32
    f32r = mybir.dt.float32r
    MUL, ADD = mybir.AluOpType.mult, mybir.AluOpType.add

    nc = bacc.Bacc(
        "TRN2", target_bir_lowering=False, debug=False, num_devices=NCORES
    )

    xpad_d = nc.dram_tensor(
        "xpad", [SPC, CIN, HP * WP], f32r, kind="ExternalInput"
    ).ap()
    # host layout: [ci, tap, e, co]
    wt_d = nc.dram_tensor(
        "wt", [CIN, NTAP * E * COUT], f32, kind="ExternalInput"
    ).ap()
    rb_d = nc.dram_tensor("rb", [128, SPC * E], f32, kind="ExternalInput").ap()
    out_d = nc.dram_tensor(
        "out", [SPC, COUT, H * W_SP], f32, kind="ExternalOutput"
    ).ap()

    TAPW = E * COUT  # 512 floats per tap in wt

    with tile.TileContext(nc) as tc:
        with (
            tc.tile_pool(name="const", bufs=1) as cst,
            tc.tile_pool(name="x", bufs=2) as xpool,
            tc.tile_pool(name="wmix", bufs=2) as wmp,
            tc.tile_pool(name="ob", bufs=3) as opool,
            tc.tile_pool(name="ps", bufs=8, space="PSUM") as pspool,
        ):
            # --- HAM warm-up: dummy matmuls on a zeroed tile during loads
            # (bf16: memset doesn't support f32r, and bf16 streams 1 cyc/row)
            zt = cst.tile([128, 512], mybir.dt.bfloat16, tag="zero")
            nc.gpsimd.memset(zt[:], 0.0)
            warm_ps = pspool.tile([128, 512], f32, tag="ps")
            for _ in range(N_WARM):
                nc.tensor.matmul(
                    warm_ps[:], zt[:, :128], zt[:], start=True, stop=True
                )

            rb_t = cst.tile([128, SPC * E], f32, tag="rb")
            nc.sync.dma_start(rb_t[:], rb_d[:])
            wt_t = cst.tile([CIN, NTAP * TAPW], f32, tag="wt")

            def load_wt_tap(t):
                sl = slice(t * TAPW, (t + 1) * TAPW)
                nc.sync.dma_start(wt_t[:, sl], wt_d[:, sl])

            def load_x_chunk(s, xtiles, c):
                r0, nr = XCH[c]
                xt = xpool.tile([CIN, nr * WP], f32r, tag=f"x{c}", name=f"x{s}_{c}")
                sl = slice(r0 * WP, (r0 + nr) * WP)
                nc.sync.dma_start(xt[:], xpad_d[s][:, sl])
                xtiles[c] = xt

            # interleaved load order for sample 0: tap weights feed the
            # tap-outer stream ASAP, x chunks arrive as taps consume rows
            x0t = [None] * 3
            load_wt_tap(0)
            load_x_chunk(0, x0t, 0)
            load_wt_tap(1)
            load_wt_tap(2)
            load_x_chunk(0, x0t, 1)
            load_wt_tap(3)
            load_wt_tap(4)
            load_x_chunk(0, x0t, 2)
            for t in range(5, NTAP):
                load_wt_tap(t)
            x1t = [None] * 3
            for c in range(3):
                load_x_chunk(1, x1t, c)

            wt3 = wt_t[:].rearrange("p (t e c) -> p t e c", t=NTAP, e=E)

            def mix(wm3, s, t0, t1):
                """wm[:, t0:t1, :] = sum_e rb[s,e] * wt[:, t0:t1, e, :]"""
                for e in range(E):
                    sc = rb_t[:, s * E + e : s * E + e + 1]
                    src = wt3[:, t0:t1, e, :]
                    dst = wm3[:, t0:t1, :]
                    if e == 0:
                        nc.vector.tensor_scalar_mul(dst, src, sc)
                    else:
                        nc.vector.scalar_tensor_tensor(
                            dst, src, sc, dst, MUL, ADD
                        )

            def rhs_ap(xtiles, blk, kh, kw):
                c = BLK_CH[blk]
                loc = blk * RPB - XCH[c][0]
                x3 = xtiles[c][:].rearrange("p (h w) -> p h w", w=WP)
                return x3[:, loc + kh : loc + kh + RPB, kw : kw + W_SP]

            def store_block(s, ob, blk):
                sl = slice(blk * NT, (blk + 1) * NT)
                nc.vector.tensor_copy(ob[:, sl], ps_map[blk][:])
                nc.scalar.dma_start(out_d[s][:, sl], ob[:, sl])

            # ---- sample 0: tap-outer over 7 live PSUM banks
            wm0 = wmp.tile([CIN, NTAP * COUT], f32r, tag="wm")
            wm0_3 = wm0[:].rearrange("p (t c) -> p t c", t=NTAP)
            ps_map = {}
            for blk in range(NBLK):
                ps_map[blk] = pspool.tile(
                    [COUT, NT], f32, tag="ps", name=f"ps0_{blk}"
                )
            for t in range(NTAP):
                # per-tap mix for the first taps (lowest latency), then in
                # 3-tap chunks
                if t < 3:
                    mix(wm0_3, 0, t, t + 1)
                elif t in (3, 6):
                    mix(wm0_3, 0, t, t + 3)
                kh, kw = divmod(t, KW)
                for blk in range(NBLK):
                    nc.tensor.matmul(
                        ps_map[blk][:],
                        wm0[:, t * COUT : (t + 1) * COUT],
                        rhs_ap(x0t, blk, kh, kw),
                        start=(t == 0),
                        stop=(t == NTAP - 1),
                        skip_group_check=True,
                    )

            # sample 1 weight mix: runs on DVE during sample 0's stream
            wm1 = wmp.tile([CIN, NTAP * COUT], f32r, tag="wm")
            wm1_3 = wm1[:].rearrange("p (t c) -> p t c", t=NTAP)
            mix(wm1_3, 1, 0, NTAP)

            # drain sample 0
            ob0 = opool.tile([COUT, H * W_SP], f32, tag="ob")
            for blk in range(NBLK):
                store_block(0, ob0, blk)

            # ---- sample 1: block-outer, drains incrementally
            ob1 = opool.tile([COUT, H * W_SP], f32, tag="ob")
            for blk in range(NBLK):
                ps = pspool.tile([COUT, NT], f32, tag="ps", name=f"ps1_{blk}")
                ps_map[blk] = ps
                for t in range(NTAP):
                    kh, kw = divmod(t, KW)
                    nc.tensor.matmul(
                        ps[:],
                        wm1[:, t * COUT : (t + 1) * COUT],
                        rhs_ap(x1t, blk, kh, kw),
                        start=(t == 0),
                        stop=(t == NTAP - 1),
                    )
                store_block(1, ob1, blk)

    nc.compile()
    return nc


def _get_nc():
    global _cached_nc
    if _cached_nc is None:
        _cached_nc = _build_nc()
    return _cached_nc


def _prep_inputs(x, routing_weights, W):
    x = np.ascontiguousarray(x, dtype=np.float32)
    routing_weights = np.ascontiguousarray(routing_weights, dtype=np.float32)
    W = np.ascontiguousarray(W, dtype=np.float32)

    xpad = np.zeros((B, CIN, HP, WP), np.float32)
    xpad[:, :, 1 : H + 1, 1 : W_SP + 1] = _round_f32r(x.reshape(B, CIN, H, W_SP))
    xpad = xpad.reshape(B, CIN, HP * WP)

    # W[e, co, ci, kh, kw] -> wt[ci, (kh, kw, e, co)]
    wt = np.ascontiguousarray(np.transpose(W, (2, 3, 4, 0, 1))).reshape(
        CIN, NTAP * E * COUT
    )

    in_maps = []
    for c in range(NCORES):
        r = routing_weights[c * SPC : (c + 1) * SPC]  # [SPC, E]
        rb = np.ascontiguousarray(
            np.broadcast_to(r.reshape(1, SPC * E), (128, SPC * E))
        )
        in_maps.append(
            {
                "xpad": xpad[c * SPC : (c + 1) * SPC],
                "wt": wt,
                "rb": rb,
            }
        )
    return in_maps


def _run(in_maps, **kwargs):
    from concourse import bass_utils

    nc = _get_nc()
    res = bass_utils.run_bass_kernel_spmd(
        nc, in_maps, core_ids=list(range(NCORES)), **kwargs
    )
    out = np.concatenate(
        [res.results[c]["out"] for c in range(NCORES)], axis=0
    ).reshape(B, COUT, H, W_SP)
    return out, res


def kernel(x, routing_weights, W):
    in_maps = _prep_inputs(x, routing_weights, W)
    out, _ = _run(in_maps)
    return out
